# revision 1
# baseline (speedup 1.0000x reference)
"""Trainium2 Bass kernel for nn_DConv (diffusion graph conv, K=2, 2 supports).

Contract: kernel(**inputs) takes FULL unsharded inputs (inputs [B,N,D] f32,
adj_vals [E] f32, rows/cols [E] int, weights [D*M,OUT] f32, biases [1,OUT]
f32) and returns the FULL output [B, N, OUT] f32.

Strategy (data-parallel over batch, per the sharding hint):
 - Each of the 8 cores handles B/8 batches: x layout [N, D*Bl] (col = d*Bl+b).
 - Host builds the two normalized supports (vals1,rows->cols / vals2,cols->rows),
   sorts each edge list by destination into 128-node blocks, pads each block's
   edge segment to a multiple of 128 "slots".
 - Device, per spmm: dma_gather (bf16, 512B rows) fetches x[src] per slot;
   a per-chunk [128,128] selection matrix Sel[e, dst_local] = v_e (built on
   the vector engine as (iota==dst)*v) reduces each chunk into PSUM via
   TensorE: y_block += Sel^T @ Z. Eviction is a plain PSUM->bf16 copy; the
   Chebyshev recurrence (x2 = 2*S*x1 - x0) is folded into the projection
   weights on the host, so the 4 spmms produce raw S-products only:
     A1 = S1 X0, R2 = S1 A1, B1 = S2 A1, R4 = S2 B1
   out = X0(W0-W2) + A1(W1-W4) + R2(2 W2) + B1 W3 + R4(2 W4) + bias.
 - Projection: DMA-transpose loads X_m^T tiles, TensorE contracts against a
   host-built block-diagonal W~ [1280, OUT*Bl].
"""
import os
import sys
import numpy as np
import ml_dtypes

for _p in ('/opt/trn_rl_repo', '/root/.axon_site/_ro/trn_rl_repo'):
    if os.path.isdir(_p) and _p not in sys.path:
        sys.path.append(_p)

import concourse.bass as bass
import concourse.mybir as mybir
import concourse.tile as tile
from concourse import bacc
from concourse.bass_utils import run_bass_kernel_spmd

BF16 = ml_dtypes.bfloat16
P = 128
NCORES = 8


# ---------------------------------------------------------------- host prep

def _build_support(vals, src, dst, n_nodes):
    """Sort edges by dst, pad each 128-node block segment to a multiple of
    128 slots. Returns slot arrays + chunk metadata."""
    nb = n_nodes // P
    order = np.argsort(dst, kind='stable')
    s_src = src[order]
    s_dst = dst[order]
    s_v = vals[order]
    blk = (s_dst // P).astype(np.int64)
    cnt = np.bincount(blk, minlength=nb)

    src_parts, dstl_parts, v_parts = [], [], []
    chunk_block = []
    pos = 0
    for b in range(nb):
        c = int(cnt[b])
        nchunk = max(1, -(-c // P))
        pad = nchunk * P - c
        src_parts.append(s_src[pos:pos + c])
        dstl_parts.append(s_dst[pos:pos + c] - b * P)
        v_parts.append(s_v[pos:pos + c])
        if pad:
            src_parts.append(np.zeros(pad, s_src.dtype))
            dstl_parts.append(np.zeros(pad, s_dst.dtype))
            v_parts.append(np.zeros(pad, np.float32))
        chunk_block += [b] * nchunk
        pos += c

    slot_src = np.concatenate(src_parts).astype(np.int16)
    slot_dstl = np.concatenate(dstl_parts).astype(np.float32)
    slot_v = np.concatenate(v_parts).astype(np.float32)
    n_chunks = len(chunk_block)

    # slot-major [128, n_chunks]: arr[p, c] = val[c*128 + p]
    dst_t = np.ascontiguousarray(slot_dstl.reshape(n_chunks, P).T)
    v_t = np.ascontiguousarray(slot_v.reshape(n_chunks, P).T)

    # wrapped idx layout [128, n_slots/16]: tile[p, j] = idx[j*16 + p%16]
    idx = slot_src.reshape(-1, 16).T  # [16, n_slots/16]
    idx_w = np.ascontiguousarray(np.tile(idx, (8, 1)))

    # chunk -> (block, first, last)
    chunk_block = np.asarray(chunk_block)
    first = np.ones(n_chunks, bool)
    first[1:] = chunk_block[1:] != chunk_block[:-1]
    last = np.ones(n_chunks, bool)
    last[:-1] = chunk_block[:-1] != chunk_block[1:]
    return dict(idx_w=idx_w, dst_t=dst_t, v_t=v_t,
                chunk_block=chunk_block, first=first, last=last,
                n_chunks=n_chunks)


def preprocess(adj_vals, rows, cols, n_nodes):
    drow = np.zeros(n_nodes, np.float32)
    np.add.at(drow, rows, adj_vals)
    dcol = np.zeros(n_nodes, np.float32)
    np.add.at(dcol, cols, adj_vals)
    inv_drow = np.where(drow > 0, 1.0 / drow, 0.0).astype(np.float32)
    inv_dcol = np.where(dcol > 0, 1.0 / dcol, 0.0).astype(np.float32)
    vals1 = (adj_vals * inv_drow[rows]).astype(np.float32)
    vals2 = (adj_vals * inv_dcol[cols]).astype(np.float32)
    s1 = _build_support(vals1, rows, cols, n_nodes)
    s2 = _build_support(vals2, cols, rows, n_nodes)
    return s1, s2


def build_wtilde(weights, d_in, n_mat, out_dim, bl):
    """W~ [5*d_in*bl, out_dim*bl] bf16 with recurrence folded in.
    Row r = m*(d_in*bl) + (d*bl + b); col = o*bl + b."""
    W = weights.reshape(d_in, n_mat, out_dim)
    C = [W[:, 0] - W[:, 2], W[:, 1] - W[:, 4], 2.0 * W[:, 2], W[:, 3], 2.0 * W[:, 4]]
    F = d_in * bl
    Wt = np.zeros((5 * F, out_dim * bl), np.float32)
    for m in range(5):
        for d in range(d_in):
            for b in range(bl):
                Wt[m * F + d * bl + b, b::bl] = C[m][d]
    return Wt.astype(BF16)


# ---------------------------------------------------------------- program

def build_program(n_nodes, feat, out_feat, sup_metas, call_chunks=64, selg=8):
    """Build the per-core Bass program. sup_metas = (s1, s2) chunk metadata
    (only n_chunks/chunk_block/first/last are used — the program layout
    depends on them)."""
    ob = 256  # out_dim * bl
    nt = n_nodes // P  # projection node tiles
    n_wchunks = 5 * feat // P

    nc = bacc.Bacc("TRN2", target_bir_lowering=False, debug=False,
                   num_devices=NCORES)
    dt = mybir.dt

    x0 = nc.dram_tensor("x0", [n_nodes, feat], dt.bfloat16, kind="ExternalInput")
    iota_in = nc.dram_tensor("iota", [P, P], dt.float32, kind="ExternalInput")
    wt_in = nc.dram_tensor("wt", [5 * feat, ob], dt.bfloat16, kind="ExternalInput")
    bias_in = nc.dram_tensor("bias", [P, ob], dt.float32, kind="ExternalInput")

    sup_t = []
    for i, s in enumerate(sup_metas):
        n_slots = s['n_chunks'] * P
        sup_t.append(dict(
            idx=nc.dram_tensor(f"idx{i}", [P, n_slots // 16], dt.int16,
                               kind="ExternalInput"),
            dst=nc.dram_tensor(f"dst{i}", [P, s['n_chunks']], dt.float32,
                               kind="ExternalInput"),
            v=nc.dram_tensor(f"v{i}", [P, s['n_chunks']], dt.float32,
                             kind="ExternalInput"),
        ))

    A1 = nc.dram_tensor("A1", [n_nodes, feat], dt.bfloat16, kind="Internal")
    R2 = nc.dram_tensor("R2", [n_nodes, feat], dt.bfloat16, kind="Internal")
    B1 = nc.dram_tensor("B1", [n_nodes, feat], dt.bfloat16, kind="Internal")
    R4 = nc.dram_tensor("R4", [n_nodes, feat], dt.bfloat16, kind="Internal")
    out = nc.dram_tensor("out", [n_nodes, ob], dt.float32, kind="ExternalOutput")

    with tile.TileContext(nc) as tc:
        with (
            tc.tile_pool(name="const", bufs=1) as cpool,
            tc.tile_pool(name="z", bufs=2) as zpool,
            tc.tile_pool(name="idx", bufs=2) as ipool,
            tc.tile_pool(name="dv", bufs=2) as dvpool,
            tc.tile_pool(name="sel", bufs=2) as selpool,
            tc.tile_pool(name="ev", bufs=4) as evpool,
            tc.tile_pool(name="lhs", bufs=2) as lpool,
            tc.tile_pool(name="po", bufs=2) as opool,
            tc.tile_pool(name="ps", bufs=4, space="PSUM") as pspool,
            tc.tile_pool(name="pso", bufs=2, space="PSUM") as psopool,
        ):
            iota_sb = cpool.tile([P, P], dt.float32)
            nc.sync.dma_start(iota_sb[:], iota_in[:, :])
            wt_sb = cpool.tile([P, n_wchunks, ob], dt.bfloat16)
            nc.sync.dma_start(
                wt_sb[:],
                wt_in[:, :].rearrange("(k p) o -> p k o", p=P))
            bias_sb = cpool.tile([P, ob], dt.float32)
            nc.sync.dma_start(bias_sb[:], bias_in[:, :])

            def emit_spmm(sup, st, xsrc, ydst):
                n_chunks = sup['n_chunks']
                cb = sup['chunk_block']
                first = sup['first']
                last = sup['last']
                ps = None
                for c0 in range(0, n_chunks, call_chunks):
                    ncall = min(call_chunks, n_chunks - c0)
                    nidx = ncall * P
                    idx_t = ipool.tile([P, call_chunks * 8], dt.int16, tag="idx")
                    nc.sync.dma_start(
                        idx_t[:, :ncall * 8],
                        st['idx'][:, c0 * 8:(c0 + ncall) * 8])
                    dst_t = dvpool.tile([P, call_chunks], dt.float32, tag="dst")
                    nc.sync.dma_start(dst_t[:, :ncall],
                                      st['dst'][:, c0:c0 + ncall])
                    v_t = dvpool.tile([P, call_chunks], dt.float32, tag="v")
                    nc.sync.dma_start(v_t[:, :ncall],
                                      st['v'][:, c0:c0 + ncall])
                    z_t = zpool.tile([P, call_chunks, feat], dt.bfloat16, tag="z")
                    nc.gpsimd.dma_gather(
                        z_t[:, :ncall, :], xsrc[:, :], idx_t[:, :ncall * 8],
                        nidx, nidx, feat, single_packet=False)
                    sel_t = selpool.tile([P, call_chunks, P], dt.bfloat16,
                                         tag="sel")
                    for g0 in range(0, ncall, selg):
                        ng = min(selg, ncall - g0)
                        sel_sl = sel_t[:, g0:g0 + ng, :]
                        nc.vector.tensor_tensor(
                            out=sel_sl,
                            in0=iota_sb[:][:, None, :].to_broadcast([P, ng, P]),
                            in1=dst_t[:, g0:g0 + ng, None].to_broadcast([P, ng, P]),
                            op=mybir.AluOpType.is_equal)
                        nc.vector.tensor_tensor(
                            out=sel_sl,
                            in0=sel_sl,
                            in1=v_t[:, g0:g0 + ng, None].to_broadcast([P, ng, P]),
                            op=mybir.AluOpType.mult)
                    for cl in range(ncall):
                        c = c0 + cl
                        if first[c]:
                            ps = pspool.tile([P, feat], dt.float32, tag="ps")
                        nc.tensor.matmul(
                            out=ps[:],
                            lhsT=sel_t[:, cl, :],
                            rhs=z_t[:, cl, :],
                            start=bool(first[c]),
                            stop=bool(last[c]),
                        )
                        if last[c]:
                            b = cb[c]
                            y_sb = evpool.tile([P, feat], dt.bfloat16, tag="y")
                            nc.vector.tensor_copy(out=y_sb[:], in_=ps[:])
                            nc.sync.dma_start(
                                ydst[b * P:(b + 1) * P, :], y_sb[:])

            emit_spmm(sup_metas[0], sup_t[0], x0, A1)
            emit_spmm(sup_metas[0], sup_t[0], A1, R2)
            emit_spmm(sup_metas[1], sup_t[1], A1, B1)
            emit_spmm(sup_metas[1], sup_t[1], B1, R4)

            # projection
            xs = [x0, A1, R2, B1, R4]
            for t in range(nt):
                rows = slice(t * P, (t + 1) * P)
                pso = psopool.tile([P, ob], dt.float32, tag="pso")
                for k in range(n_wchunks):
                    m, h = divmod(k, feat // P)
                    lhsT = lpool.tile([P, P], dt.bfloat16, tag="lhsT")
                    nc.sync.dma_start_transpose(
                        lhsT[:], xs[m][rows, h * P:(h + 1) * P])
                    nc.tensor.matmul(
                        out=pso[:],
                        lhsT=lhsT[:],
                        rhs=wt_sb[:, k, :],
                        start=(k == 0),
                        stop=(k == n_wchunks - 1),
                    )
                o_sb = opool.tile([P, ob], dt.float32, tag="osb")
                nc.vector.tensor_tensor(out=o_sb[:], in0=pso[:],
                                        in1=bias_sb[:],
                                        op=mybir.AluOpType.add)
                nc.sync.dma_start(out[rows, :], o_sb[:])

    nc.compile()
    return nc


# ---------------------------------------------------------------- entry

def _make_core_inputs(core, inputs_f32, s1, s2, wt, bias_rep, n_nodes, d_in):
    bl = inputs_f32.shape[0] // NCORES
    x0 = np.ascontiguousarray(
        inputs_f32[core * bl:(core + 1) * bl]
        .transpose(1, 2, 0).reshape(n_nodes, d_in * bl)).astype(BF16)
    iota = np.tile(np.arange(P, dtype=np.float32)[None, :], (P, 1))
    return dict(
        x0=x0, iota=iota, wt=wt, bias=bias_rep,
        idx0=s1['idx_w'], dst0=s1['dst_t'], v0=s1['v_t'],
        idx1=s2['idx_w'], dst1=s2['dst_t'], v1=s2['v_t'],
    )


def kernel(**inputs):
    inputs_f32 = np.asarray(inputs['inputs'], dtype=np.float32)
    adj_vals = np.asarray(inputs['adj_vals'], dtype=np.float32)
    rows = np.asarray(inputs['rows']).astype(np.int64)
    cols = np.asarray(inputs['cols']).astype(np.int64)
    weights = np.asarray(inputs['weights'], dtype=np.float32)
    biases = np.asarray(inputs['biases'], dtype=np.float32)

    b_total, n_nodes, d_in = inputs_f32.shape
    out_dim = weights.shape[1]
    n_mat = weights.shape[0] // d_in
    bl = b_total // NCORES
    assert n_mat == 5, "kernel is specialized for K=2 (M=5)"

    s1, s2 = preprocess(adj_vals, rows, cols, n_nodes)
    wt = build_wtilde(weights, d_in, n_mat, out_dim, bl)
    bias_rep = np.zeros((P, out_dim * bl), np.float32)
    for o in range(out_dim):
        bias_rep[:, o * bl:(o + 1) * bl] = biases[0, o]

    nc = build_program(n_nodes, d_in * bl, out_dim, (s1, s2))

    in_maps = [
        _make_core_inputs(c, inputs_f32, s1, s2, wt, bias_rep, n_nodes, d_in)
        for c in range(NCORES)
    ]
    res = run_bass_kernel_spmd(nc, in_maps, core_ids=list(range(NCORES)))

    out = np.zeros((b_total, n_nodes, out_dim), np.float32)
    for c in range(NCORES):
        oc = res.results[c]['out']  # [n_nodes, out*bl], col = o*bl + b
        out[c * bl:(c + 1) * bl] = (
            oc.reshape(n_nodes, out_dim, bl).transpose(2, 0, 1))
    return out



# revision 2
# speedup vs baseline: 1.1762x; 1.1762x over previous
"""Trainium2 Bass kernel v2 for nn_DConv — 8-way NODE sharding.

Each core owns 2048 dst nodes (16 blocks of 128); feat = D*B = 2048 bf16
columns (batch-major: f = b*64 + d).  Per spmm a core gathers only its own
65536 edges (4KB rows) — 8x fewer SWDGE descriptors than batch-DP, which was
the baseline bottleneck (GPSIMD desc-gen at ~7.8ns/desc, 71% busy).

Levels: A1 = S1 X0, R2 = S1 A1, B1 = S2 A1, R4 = S2 B1.  X0/A1/B1 must be
full tensors on every core (gather sources) -> HBM AllGather collectives
between levels; R2/R4 stay local (projection-only).

Projection: per-block PE transposes produce xT [feat, nodes] tiles in HBM;
out^T[o, n] accumulates sum_m C_m^T @ xT_m[d-range(b), :] per batch with the
Chebyshev recurrence folded into C_m; bias added on the Act engine.
"""
import os
import sys
import numpy as np
import ml_dtypes

for _p in ('/opt/trn_rl_repo', '/root/.axon_site/_ro/trn_rl_repo'):
    if os.path.isdir(_p) and _p not in sys.path:
        sys.path.append(_p)

import concourse.bass as bass
import concourse.mybir as mybir
import concourse.tile as tile
from concourse import bacc
from concourse.bass_utils import run_bass_kernel_spmd

BF16 = ml_dtypes.bfloat16
P = 128
NCORES = 8
N = 16384
D = 64
B = 32
FEAT = D * B          # 2048 (f = b*64 + d)
NPC = N // NCORES     # 2048 nodes per core
NBLK = NPC // P       # 16 blocks per core
CALL = 8              # chunks per gather call
TBLK = 4              # blocks staged per xT write
NG = 256              # projection node-group size
OUT = 64


# ---------------------------------------------------------------- host prep

def preprocess_oct(vals, src, dst, n_nodes):
    """Per-core slot arrays, uniform CPB chunks per (core, block).
    Within each dst block, edges sorted by src for HBM gather locality."""
    order = np.argsort(dst, kind='stable')
    s_src = src[order]
    s_dst = dst[order]
    s_v = vals[order]
    nb = n_nodes // P
    blk = (s_dst // P).astype(np.int64)
    cnt = np.bincount(blk, minlength=nb)
    cpb = int(np.max((cnt + P - 1) // P))
    pos = np.concatenate([[0], np.cumsum(cnt)])
    cores = []
    for c in range(NCORES):
        srcs = np.zeros((NBLK, cpb * P), np.int16)
        dsts = np.zeros((NBLK, cpb * P), np.float32)
        vs = np.zeros((NBLK, cpb * P), np.float32)
        for b in range(NBLK):
            g = c * NBLK + b
            lo, hi = pos[g], pos[g + 1]
            e_src = s_src[lo:hi]
            e_dst = s_dst[lo:hi] - g * P
            e_v = s_v[lo:hi]
            o2 = np.argsort(e_src, kind='stable')
            n_e = hi - lo
            srcs[b, :n_e] = e_src[o2]
            dsts[b, :n_e] = e_dst[o2]
            vs[b, :n_e] = e_v[o2]
        slot_src = srcs.reshape(-1)
        nch = NBLK * cpb
        idx_w = np.ascontiguousarray(np.tile(slot_src.reshape(-1, 16).T, (8, 1)))
        dst_t = np.ascontiguousarray(dsts.reshape(nch, P).T)
        v_t = np.ascontiguousarray(vs.reshape(nch, P).T)
        cores.append(dict(idx=idx_w, dst=dst_t, v=v_t))
    return cores, cpb


def preprocess(adj_vals, rows, cols, n_nodes):
    drow = np.zeros(n_nodes, np.float32)
    np.add.at(drow, rows, adj_vals)
    dcol = np.zeros(n_nodes, np.float32)
    np.add.at(dcol, cols, adj_vals)
    inv_drow = np.where(drow > 0, 1.0 / drow, 0.0).astype(np.float32)
    inv_dcol = np.where(dcol > 0, 1.0 / dcol, 0.0).astype(np.float32)
    vals1 = (adj_vals * inv_drow[rows]).astype(np.float32)
    vals2 = (adj_vals * inv_dcol[cols]).astype(np.float32)
    s1, cpb1 = preprocess_oct(vals1, rows, cols, n_nodes)
    s2, cpb2 = preprocess_oct(vals2, cols, rows, n_nodes)
    return s1, cpb1, s2, cpb2


# ---------------------------------------------------------------- program

def build_program(cpb1, cpb2):
    nc = bacc.Bacc("TRN2", target_bir_lowering=False, debug=False,
                   num_devices=NCORES)
    dt = mybir.dt
    groups = [list(range(NCORES))]

    xsl = nc.dram_tensor("xsl", [NPC, FEAT], dt.bfloat16, kind="ExternalInput")
    x0t = nc.dram_tensor("x0t", [P, FEAT // P, NPC], dt.bfloat16,
                         kind="ExternalInput")
    iota_in = nc.dram_tensor("iota", [P, P], dt.float32, kind="ExternalInput")
    ident_in = nc.dram_tensor("ident", [P, P], dt.bfloat16, kind="ExternalInput")
    cw_in = nc.dram_tensor("cw", [P, 5, OUT], dt.bfloat16, kind="ExternalInput")
    bias_in = nc.dram_tensor("bias", [64, 1], dt.float32, kind="ExternalInput")
    sup_in = []
    for i, cpb in ((0, cpb1), (1, cpb2)):
        nch = NBLK * cpb
        sup_in.append(dict(
            idx=nc.dram_tensor(f"idx{i}", [P, nch * 8], dt.int16,
                               kind="ExternalInput"),
            dst=nc.dram_tensor(f"dst{i}", [P, nch], dt.float32,
                               kind="ExternalInput"),
            v=nc.dram_tensor(f"v{i}", [P, nch], dt.float32,
                             kind="ExternalInput"),
        ))

    Xb = nc.dram_tensor("Xb", [NPC, FEAT], dt.bfloat16, kind="Internal")
    X0 = nc.dram_tensor("X0", [N, FEAT], dt.bfloat16, kind="Internal")
    A1 = nc.dram_tensor("A1", [N, FEAT], dt.bfloat16, kind="Internal")
    B1 = nc.dram_tensor("B1", [N, FEAT], dt.bfloat16, kind="Internal")
    Asl = nc.dram_tensor("Asl", [NPC, FEAT], dt.bfloat16, kind="Internal")
    Bsl = nc.dram_tensor("Bsl", [NPC, FEAT], dt.bfloat16, kind="Internal")
    aT = nc.dram_tensor("aT", [P, FEAT // P, NPC], dt.bfloat16, kind="Internal")
    r2T = nc.dram_tensor("r2T", [P, FEAT // P, NPC], dt.bfloat16, kind="Internal")
    bT = nc.dram_tensor("bT", [P, FEAT // P, NPC], dt.bfloat16, kind="Internal")
    r4T = nc.dram_tensor("r4T", [P, FEAT // P, NPC], dt.bfloat16, kind="Internal")
    outT = nc.dram_tensor("outT", [OUT, B, NPC], dt.float32,
                          kind="ExternalOutput")

    with tile.TileContext(nc) as tc:
        with (
            tc.tile_pool(name="const", bufs=1) as cpool,
            tc.tile_pool(name="z", bufs=2) as zpool,
            tc.tile_pool(name="sel", bufs=2) as selpool,
            tc.tile_pool(name="y", bufs=2) as ypool,
            tc.tile_pool(name="xst", bufs=2) as xstpool,
            tc.tile_pool(name="slab", bufs=1) as slabpool,
            tc.tile_pool(name="o", bufs=3) as opool,
            tc.tile_pool(name="ps", bufs=1, space="PSUM") as pspool,
            tc.tile_pool(name="tp", bufs=2, space="PSUM") as tpool,
            tc.tile_pool(name="po", bufs=2, space="PSUM") as popool,
        ):
            iota_sb = cpool.tile([P, P], dt.float32)
            nc.sync.dma_start(iota_sb[:], iota_in[:, :])
            ident_sb = cpool.tile([P, P], dt.bfloat16)
            nc.sync.dma_start(ident_sb[:], ident_in[:, :])
            cw_sb = cpool.tile([P, 5, OUT], dt.bfloat16)
            nc.sync.dma_start(cw_sb[:], cw_in[:, :, :])
            bias_sb = cpool.tile([64, 1], dt.float32)
            nc.sync.dma_start(bias_sb[:], bias_in[:, :])
            sup_sb = []
            for i, cpb in ((0, cpb1), (1, cpb2)):
                nch = NBLK * cpb
                idx_sb = cpool.tile([P, nch * 8], dt.int16)
                nc.sync.dma_start(idx_sb[:], sup_in[i]['idx'][:, :])
                dst_sb = cpool.tile([P, nch], dt.float32)
                nc.sync.dma_start(dst_sb[:], sup_in[i]['dst'][:, :])
                v_sb = cpool.tile([P, nch], dt.float32)
                nc.sync.dma_start(v_sb[:], sup_in[i]['v'][:, :])
                sup_sb.append(dict(idx=idx_sb, dst=dst_sb, v=v_sb))

            def allgather(src, dst):
                nc.gpsimd.collective_compute(
                    "AllGather", mybir.AluOpType.bypass,
                    replica_groups=groups,
                    ins=[src[:, :]], outs=[dst[:, :]])

            def emit_spmm(sup, cpb, xsrc, ysl, yT):
                n_chunks = NBLK * cpb
                ps = None
                xstage = None
                for c0 in range(0, n_chunks, CALL):
                    z = zpool.tile([P, CALL, FEAT], dt.bfloat16, tag="z")
                    nc.gpsimd.dma_gather(
                        z[:, :, :], xsrc[:, :],
                        sup['idx'][:, c0 * 8:(c0 + CALL) * 8],
                        CALL * P, CALL * P, FEAT, single_packet=False)
                    sel = selpool.tile([P, CALL, P], dt.bfloat16, tag="sel")
                    nc.vector.tensor_tensor(
                        out=sel[:],
                        in0=iota_sb[:][:, None, :].to_broadcast([P, CALL, P]),
                        in1=sup['dst'][:, c0:c0 + CALL, None]
                            .to_broadcast([P, CALL, P]),
                        op=mybir.AluOpType.is_equal)
                    nc.vector.tensor_tensor(
                        out=sel[:], in0=sel[:],
                        in1=sup['v'][:, c0:c0 + CALL, None]
                            .to_broadcast([P, CALL, P]),
                        op=mybir.AluOpType.mult)
                    for cl in range(CALL):
                        c = c0 + cl
                        if c % cpb == 0:
                            ps = pspool.tile([P, FEAT], dt.float32, tag="ps")
                        for g in range(FEAT // 512):
                            nc.tensor.matmul(
                                out=ps[:, g * 512:(g + 1) * 512],
                                lhsT=sel[:, cl, :],
                                rhs=z[:, cl, g * 512:(g + 1) * 512],
                                start=(c % cpb == 0),
                                stop=(c % cpb == cpb - 1))
                        if c % cpb == cpb - 1:
                            b = c // cpb
                            y_sb = ypool.tile([P, FEAT], dt.bfloat16, tag="y")
                            nc.scalar.copy(out=y_sb[:], in_=ps[:])
                            if ysl is not None:
                                nc.sync.dma_start(
                                    ysl[b * P:(b + 1) * P, :], y_sb[:])
                            q = b % TBLK
                            if q == 0:
                                xstage = xstpool.tile(
                                    [P, FEAT // P, TBLK * P], dt.bfloat16,
                                    tag="xst")
                            for r in range(FEAT // P):
                                tp = tpool.tile([P, P], dt.bfloat16, tag="tp")
                                nc.tensor.transpose(
                                    tp[:], y_sb[:, r * P:(r + 1) * P],
                                    ident_sb[:])
                                nc.scalar.copy(
                                    out=xstage[:, r, q * P:(q + 1) * P],
                                    in_=tp[:])
                            if q == TBLK - 1:
                                nc.sync.dma_start(
                                    yT[:, :, (b - TBLK + 1) * P:(b + 1) * P],
                                    xstage[:])

            nc.sync.dma_start(Xb[:, :], xsl[:, :])
            allgather(Xb, X0)
            emit_spmm(sup_sb[0], cpb1, X0, Asl, aT)
            allgather(Asl, A1)
            emit_spmm(sup_sb[0], cpb1, A1, None, r2T)
            emit_spmm(sup_sb[1], cpb2, A1, Bsl, bT)
            allgather(Bsl, B1)
            emit_spmm(sup_sb[1], cpb2, B1, None, r4T)

            # projection: out^T[o, n] per batch = sum_m C_m^T @ xT_m[d(b), n]
            xTs = [x0t, aT, r2T, bT, r4T]
            for gi in range(NPC // NG):
                slabs = []
                for t, xT in enumerate(xTs):
                    sl = slabpool.tile([P, FEAT // P, NG], dt.bfloat16,
                                       tag=f"slab{t}")
                    nc.sync.dma_start(sl[:], xT[:, :, gi * NG:(gi + 1) * NG])
                    slabs.append(sl)
                for b in range(B):
                    po = popool.tile([64, NG], dt.float32, tag="po")
                    for m in range(5):
                        nc.tensor.matmul(
                            out=po[:],
                            lhsT=cw_sb[(b % 2) * 64:(b % 2) * 64 + 64, m, :],
                            rhs=slabs[m][(b % 2) * 64:(b % 2) * 64 + 64,
                                         b // 2, :],
                            start=(m == 0), stop=(m == 4))
                    o_sb = opool.tile([64, NG], dt.float32, tag="o")
                    nc.scalar.activation(
                        out=o_sb[:], in_=po[:],
                        func=mybir.ActivationFunctionType.Identity,
                        bias=bias_sb[:, 0:1], scale=1.0)
                    nc.sync.dma_start(
                        outT[:, b, gi * NG:(gi + 1) * NG], o_sb[:])

    nc.compile()
    return nc


# ---------------------------------------------------------------- entry

def make_core_inputs(core, inputs_f32, s1, s2, cw, bias):
    # x slice [NPC, FEAT] with f = b*64 + d (batch-major)
    xc = inputs_f32[:, core * NPC:(core + 1) * NPC, :]    # [B, NPC, D]
    xsl = np.ascontiguousarray(
        xc.transpose(1, 0, 2).reshape(NPC, FEAT)).astype(BF16)
    x0t = np.ascontiguousarray(
        xsl.reshape(NPC, FEAT // P, P).transpose(2, 1, 0)).astype(BF16)
    iota = np.tile(np.arange(P, dtype=np.float32)[None, :], (P, 1))
    ident = np.eye(P, dtype=np.float32).astype(BF16)
    return dict(
        xsl=xsl, x0t=x0t, iota=iota, ident=ident, cw=cw, bias=bias,
        idx0=s1[core]['idx'], dst0=s1[core]['dst'], v0=s1[core]['v'],
        idx1=s2[core]['idx'], dst1=s2[core]['dst'], v1=s2[core]['v'],
    )


def prepare(inputs):
    inputs_f32 = np.asarray(inputs['inputs'], dtype=np.float32)
    adj_vals = np.asarray(inputs['adj_vals'], dtype=np.float32)
    rows = np.asarray(inputs['rows']).astype(np.int64)
    cols = np.asarray(inputs['cols']).astype(np.int64)
    weights = np.asarray(inputs['weights'], dtype=np.float32)
    biases = np.asarray(inputs['biases'], dtype=np.float32)

    s1, cpb1, s2, cpb2 = preprocess(adj_vals, rows, cols, N)

    W = weights.reshape(D, 5, OUT)
    C = np.stack([W[:, 0] - W[:, 2], W[:, 1] - W[:, 4], 2.0 * W[:, 2],
                  W[:, 3], 2.0 * W[:, 4]], axis=1)  # [64, 5, 64]
    cw = np.ascontiguousarray(np.concatenate([C, C], axis=0)).astype(BF16)
    bias = np.ascontiguousarray(biases.reshape(OUT, 1)).astype(np.float32)

    in_maps = [make_core_inputs(c, inputs_f32, s1, s2, cw, bias)
               for c in range(NCORES)]
    return in_maps, cpb1, cpb2


def assemble(res):
    out = np.zeros((B, N, OUT), np.float32)
    for c in range(NCORES):
        oc = res.results[c]['outT']  # [OUT, B, NPC]
        out[:, c * NPC:(c + 1) * NPC, :] = oc.transpose(1, 2, 0)
    return out


def kernel(**inputs):
    in_maps, cpb1, cpb2 = prepare(inputs)
    nc = build_program(cpb1, cpb2)
    res = run_bass_kernel_spmd(nc, in_maps, core_ids=list(range(NCORES)))
    return assemble(res)
